# revision 22
# baseline (speedup 1.0000x reference)
"""CrossScanAttention (bimamba-v3) Trainium2 kernel — v2.

Full inputs -> shard batch across 8 NeuronCores (2 batches/core) -> full output.
Self-contained: hardcodes all shapes; no sibling imports, no file reads.

v2 design:
  The two local batches are interleaved element-by-element along the sequence
  (col = 2*t + b), so every elementwise/matmul stage processes fused
  [128, 1536] tiles, and the selective scan becomes a distance-2 recurrence
  handled by a hand-authored custom DVE op (SCAN_I2_ANT) at ~1 elem/cycle —
  2x the stock tensor_tensor_scan's rate. The scan runs per state-quarter
  (4 states) in-place over the dA tile; y = sum_s C_s*h_s accumulates
  incrementally to free quarters early. dA powers exploit A_s = (s+1)*A_0:
  q = exp(A_0*dt), then 4 ACT squares + 11 DVE muls build q^1..q^16.
  u = dt*xs and the z-gates run on the (otherwise idle) GPSIMD engine.
  ACT ops are emitted function-set-major (logexp -> silu -> logexp ->
  sigmoid) to avoid activation-table thrash.
"""

import numpy as np

# ---- problem constants ----
B, C, H, W = 16, 768, 32, 32
D_MODEL, D_INNER, D_STATE, DT_RANK, D_CONV = 64, 128, 16, 4, 4
LN_EPS = 1e-5
NCORES = 8
BL = B // NCORES          # 2 local batches per core, interleaved
L = C                     # 768
L2 = 2 * L                # 1536 interleaved data cols
TS2 = L2 + 4              # 1540: + 2 zero break-cols per interleaved stream
NQ = 4                    # states per scan quarter

_cached = {}

# p128f column layout
PF_A, PF_CW, PF_CB, PF_DTB, PF_D, PF_V, PF_BE = 0, 48, 60, 63, 66, 69, 70
PF_NCOL = 76

SCAN_I2_ROW = 17  # rows 1..16 used by stock OPS; [1, 0x20) free


def _make_scan_i2():
    """Hand-authored custom DVE op: h[i] = a[i]*h[i-2] + b[i] (h[-2:]=0).

    Two interleaved recurrences at 1 elem/cycle: blk0 multiplies the a-stream
    by blk1's a_flop (which by register timing holds h from exactly 2 elements
    back), blk1 adds the b-stream and refreshes a_flop. A 2-slot priming uOp
    zeroes a_flop first.
    """
    import concourse.dve_ops as dve_ops
    from concourse.dve_spec import Spec, Src0, Src1, Bin
    from concourse.dve_uop import (
        UopConfig, UopDpConfig, DveOpSpec, InpSel, OutSel, OutPath, AluOp,
        AluInp, DelayInp, Trigger, ENABLE,
    )

    def build_uops():
        prime = UopConfig()
        prime.enable_input(InpSel.ZERO, 3)
        dp = [UopDpConfig() for _ in range(8)]
        dp[0].enable_alu(AluOp.BYPASS, AluInp.PREV_DELAY_2)
        dp[1].enable_alu(AluOp.BYPASS, AluInp.PREV_ALU_OUT)
        dp[1].alu_out_a_enable = ENABLE
        for k in range(2, 8):
            dp[k].pass_through_alu()
        prime.datapath_config = dp
        prime.repeat_count = 2
        prime.trigger = (Trigger.COUNT, Trigger.NONE, Trigger.NONE)
        prime.next_uop = (1, 0, 0)

        steady = UopConfig()
        steady.enable_input(InpSel.SRC_0, 1)
        steady.enable_input(InpSel.SRC_1, 2)
        dp = [UopDpConfig() for _ in range(8)]
        dp[0].enable_alu(AluOp.MULTIPLY, AluInp.PREV_DELAY_0,
                         AluInp.NEXT_ALU_OUT_A)
        dp[0].enable_delay_from_src(DelayInp.PREV_DELAY, 1)
        dp[1].enable_alu(AluOp.ADD, AluInp.PREV_ALU_OUT, AluInp.PREV_DELAY_1)
        dp[1].alu_out_a_enable = ENABLE
        for k in range(2, 8):
            dp[k].pass_through_alu()
        steady.datapath_config = dp
        steady.require_inp0 = 1
        steady.require_inp1 = 1
        steady.enable_output(OutSel.ALU_OUT, OutPath.WR0_LO)
        steady.trigger = (Trigger.SRC_TENSOR_DONE, Trigger.NONE, Trigger.NONE)
        steady.next_uop = (0, 0, 0)
        return [prime, steady]

    class ScanI2Op:
        name = "SCAN_I2_ANT"
        # dummy body for _custom_dve's shape asserts (reads Src1, no C2);
        # real semantics come from the hand uop table.
        spec = Spec(body=Bin(AluOp.ADD, Bin(AluOp.MULTIPLY, Src0, Src1), Src1),
                    reference=lambda in0, in1, s0, s1, imm2: None)
        subdim = False
        perf_en = {}

        def compile(self, ver):
            return DveOpSpec(name=self.name, opcode=SCAN_I2_ROW,
                             uops=build_uops(), rd1_en=True)

    op = ScanI2Op()
    if op.name not in dve_ops._SUB_OPCODE_FOR_NAME:
        dve_ops.OPS.append(op)
        dve_ops._SUB_OPCODE_FOR_NAME[op.name] = SCAN_I2_ROW
        dve_ops.CUSTOM_DVE_SPECS[op.name] = op.spec
    return op


def _build_nc():
    import concourse.bass as bass
    import concourse.bacc as bacc
    import concourse.tile as tile
    import concourse.mybir as mybir
    from concourse.masks import make_identity
    from contextlib import ExitStack

    SCAN_I2 = _make_scan_i2()

    f32 = mybir.dt.float32
    bf16 = mybir.dt.bfloat16
    AL = mybir.AluOpType
    AF = mybir.ActivationFunctionType
    AX = mybir.AxisListType

    nc = bacc.Bacc("TRN2", target_bir_lowering=False, debug=False)

    img1 = nc.dram_tensor("img1", (BL, C, H, W), f32, kind="ExternalInput").ap()
    img2 = nc.dram_tensor("img2", (BL, C, H, W), f32, kind="ExternalInput").ap()
    p128f = nc.dram_tensor("p128f", (128, PF_NCOL), f32, kind="ExternalInput").ap()
    p128b = nc.dram_tensor("p128b", (128, 108), bf16, kind="ExternalInput").ap()
    p17f = nc.dram_tensor("p17f", (17, 64), bf16, kind="ExternalInput").ap()
    p4b = nc.dram_tensor("p4b", (4, 384), bf16, kind="ExternalInput").ap()
    p64b = nc.dram_tensor("p64b", (64, 512), bf16, kind="ExternalInput").ap()
    miscf = nc.dram_tensor("miscf", (1, 2), f32, kind="ExternalInput").ap()
    att_out = nc.dram_tensor("att", (1, L2), bf16, kind="ExternalOutput").ap()

    def view3(ap):
        """[p, 1536] -> [p, 768, 2] (t, batch)."""
        return ap.rearrange("p (t e) -> p t e", e=2)

    def rev2(ap):
        """[p, 1536] -> [p, 768, 2] with t reversed (batch lanes kept)."""
        steps = [list(x) for x in ap.ap]
        st, n = steps[-1]
        assert n == L2
        newap = steps[:-1] + [[-2 * st, L], [st, 2]]
        return bass.AP(tensor=ap.tensor, offset=ap.offset + st * (L2 - 2),
                       ap=newap)

    def stride2(ap, col0, n):
        """[p, X] -> [p, n] at stride 2 starting from col0."""
        steps = [list(x) for x in ap.ap]
        st, _ = steps[-1]
        newap = steps[:-1] + [[2 * st, n]]
        return bass.AP(tensor=ap.tensor, offset=ap.offset + st * col0, ap=newap)

    with nc.allow_low_precision("bf16 intermediate precision is sufficient"), \
         tile.TileContext(nc) as tc, ExitStack() as ctx:
        consts = ctx.enter_context(tc.tile_pool(name="consts", bufs=1))
        imgp = ctx.enter_context(tc.tile_pool(name="imgp", bufs=3))
        small_ps = ctx.enter_context(tc.tile_pool(name="small_ps", bufs=2, space="PSUM"))
        mm_ps = ctx.enter_context(tc.tile_pool(name="mm_ps", bufs=2, space="PSUM"))
        stats = ctx.enter_context(tc.tile_pool(name="stats", bufs=2))
        xtp = ctx.enter_context(tc.tile_pool(name="xtp", bufs=2))
        szp = ctx.enter_context(tc.tile_pool(name="szp", bufs=2))
        xpadp = ctx.enter_context(tc.tile_pool(name="xpadp", bufs=2))
        xsp = ctx.enter_context(tc.tile_pool(name="xsp", bufs=3))
        dbcp = ctx.enter_context(tc.tile_pool(name="dbcp", bufs=2))
        dtp = ctx.enter_context(tc.tile_pool(name="dtp", bufs=2))
        up = ctx.enter_context(tc.tile_pool(name="up", bufs=2))
        dAp = ctx.enter_context(tc.tile_pool(name="dAp", bufs=5))
        dBup = ctx.enter_context(tc.tile_pool(name="dBup", bufs=1))
        bcp = ctx.enter_context(tc.tile_pool(name="bcp", bufs=3))
        seqp = ctx.enter_context(tc.tile_pool(name="seqp", bufs=2))
        ysp = ctx.enter_context(tc.tile_pool(name="ysp", bufs=1))
        gatep = ctx.enter_context(tc.tile_pool(name="gatep", bufs=1))
        dramp = ctx.enter_context(tc.tile_pool(name="dramp", bufs=2, space="DRAM"))
        outp = ctx.enter_context(tc.tile_pool(name="outp", bufs=1))

        # ---- constants ----
        c128f = consts.tile([128, PF_NCOL], f32)
        nc.sync.dma_start(out=c128f, in_=p128f)
        c128b = consts.tile([128, 108], bf16)
        nc.sync.dma_start(out=c128b, in_=p128b)
        c17 = consts.tile([16, 64], bf16)
        nc.sync.dma_start(out=c17, in_=p17f[0:16, :])
        c_preb = consts.tile([1, 64], bf16)
        nc.sync.dma_start(out=c_preb, in_=p17f[16:17, :])
        ones1 = consts.tile([1, 128], bf16)
        nc.vector.memset(ones1[:, :], 1.0)
        c4 = consts.tile([4, 384], bf16)
        nc.sync.dma_start(out=c4, in_=p4b)
        c64 = consts.tile([64, 512], bf16)
        nc.sync.dma_start(out=c64, in_=p64b)
        cmisc = consts.tile([1, 2], f32)
        nc.sync.dma_start(out=cmisc, in_=miscf)
        vcolb = consts.tile([128, 1], bf16)
        nc.scalar.copy(out=vcolb[:, :], in_=c128f[:, PF_V:PF_V + 1])
        ident = consts.tile([128, 128], f32)
        make_identity(nc, ident[:, :])
        identb = consts.tile([128, 128], bf16)
        make_identity(nc, identb[:, :])

        # ---- stage A: pool -> pooledT [16, 4*768] bf16 ----
        pooledT = outp.tile([16, 4 * L], bf16)
        imgs = [img1, img2]
        for i in range(2):
            for b in range(BL):
                for k in range(6):
                    it = imgp.tile([128, 1024], f32, tag="imgtile")
                    src = imgs[i][b, k * 128:(k + 1) * 128, :, :].rearrange(
                        "c h w -> c (h w)")
                    (nc.sync if i == 0 else nc.scalar).dma_start(out=it, in_=src)
                    v5 = it[:, :].rearrange("p (jr hb jc wb) -> p jr jc hb wb",
                                            jr=4, hb=8, jc=4, wb=8)
                    psum = stats.tile([128, 16], bf16, tag="poolsum")
                    nc.vector.tensor_reduce(out=psum, in_=v5, axis=AX.XY, op=AL.add)
                    pt_ps = small_ps.tile([16, 128], bf16, tag="sm")
                    nc.tensor.transpose(pt_ps[:, :], psum[:, :], identb[:, :])
                    col = (i * BL + b) * L + k * 128
                    nc.scalar.copy(out=pooledT[0:16, col:col + 128], in_=pt_ps)

        # ---- stage B: pre-proj + LN -> xt2[i] [64, 1536] bf16 (interleaved) ----
        xt2 = {}
        for i in range(2):
            xt2[i] = xtp.tile([64, L2], bf16, tag="xt2", name=f"xt2_{i}")

        def stage_b(i):
            for b in range(BL):
                for k in range(6):
                    col = (i * BL + b) * L + k * 128
                    xp_ps = small_ps.tile([128, 64], f32, tag="sm")
                    nc.tensor.matmul(xp_ps[:, :], lhsT=pooledT[:, col:col + 128],
                                     rhs=c17[:, :], start=True, stop=False)
                    nc.tensor.matmul(xp_ps[:, :], lhsT=ones1[:, :],
                                     rhs=c_preb[:, :], start=False, stop=True)
                    st6 = stats.tile([128, 6], f32, tag="bnst")
                    nc.vector.bn_stats(out=st6, in_=xp_ps)
                    mv = stats.tile([128, 2], f32, tag="bnmv")
                    nc.vector.bn_aggr(out=mv, in_=st6)
                    sq = stats.tile([128, 1], f32, tag="sq")
                    nc.scalar.activation(sq[:, :], mv[:, 1:2], AF.Sqrt,
                                         bias=c128f[:, 74:75])
                    rs = stats.tile([128, 1], f32, tag="rs")
                    nc.vector.reciprocal(out=rs[:, :], in_=sq[:, :])
                    xn = stats.tile([128, 64], f32, tag="xn")
                    nc.vector.tensor_scalar(out=xn[:, :], in0=xp_ps[:, :],
                                            scalar1=mv[:, 0:1], scalar2=rs[:, 0:1],
                                            op0=AL.subtract, op1=AL.mult)
                    xn_ps = small_ps.tile([64, 128], f32, tag="sm")
                    nc.tensor.transpose(xn_ps[:, :], xn[:, :], ident[:, :])
                    # interleaved write: batch b -> columns 2t+b
                    nc.scalar.copy(out=stride2(xt2[i][:, :], 256 * k + b, 128),
                                   in_=xn_ps)

        # ---- stage C: in_proj -> xpad[br] [128, 1542] bf16, z[i] [128,1536] ----
        xpad = {}
        z = {}

        def stage_c(i):
            for h in range(2):
                ps = mm_ps.tile([128, L2], f32, tag="mm")
                wsl = c64[:, i * 256 + h * 128: i * 256 + (h + 1) * 128]
                for c0 in range(0, L2, 512):
                    nc.tensor.matmul(ps[:, c0:c0 + 512], lhsT=wsl,
                                     rhs=xt2[i][:, c0:c0 + 512],
                                     start=True, stop=True)
                be = c128f[:, PF_BE + i * 2 + h: PF_BE + i * 2 + h + 1]
                if h == 0:
                    brs = [0, 1] if i == 0 else [2]
                    for br in brs:
                        xp = xpadp.tile([128, L2 + 6], bf16, tag="xpad")
                        nc.vector.memset(xp[:, 0:6], 0.0)
                        if br == 1:
                            nc.scalar.activation(
                                view3(xp[:, 6:L2 + 6]), rev2(ps[:, :]),
                                AF.Identity, bias=be)
                        else:
                            nc.scalar.activation(xp[:, 6:L2 + 6], ps[:, :],
                                                 AF.Identity, bias=be)
                        xpad[br] = xp
                else:
                    zt = szp.tile([128, L2], bf16, tag="z")
                    nc.scalar.activation(zt[:, :], ps[:, :], AF.Silu, bias=be)
                    z[i] = zt

        # ---- stage D1a: conv (DVE STT) + silu (ACT, silu set) ----
        xs_map = {}

        def d1a(br):
            xp = xpad[br]
            cw = c128f[:, PF_CW + br * 4: PF_CW + (br + 1) * 4]
            cacc = seqp.tile([128, L2], bf16, tag="cacc")
            nc.vector.tensor_scalar_mul(cacc[:, :], xp[:, 0:L2], cw[:, 0:1])
            for k in range(1, 4):
                nc.vector.scalar_tensor_tensor(
                    out=cacc[:, :], in0=xp[:, 2 * k:L2 + 2 * k],
                    scalar=cw[:, k:k + 1],
                    in1=cacc[:, :], op0=AL.mult, op1=AL.add)
            xs = xsp.tile([128, L2], bf16, tag="xs")
            nc.scalar.activation(xs[:, :], cacc[:, :], AF.Silu,
                                 bias=c128f[:, PF_CB + br:PF_CB + br + 1])
            xs_map[br] = xs

        # interleaved front-end schedule: image 0's projection + convs
        # overlap image 1's LayerNorm
        stage_b(0)
        stage_c(0)
        d1a(0)
        d1a(1)
        stage_b(1)
        stage_c(1)
        d1a(2)

        # ---- stage D1b: xproj, dtproj, dt chain, u, bc staging ----
        # softplus = ln(1 + exp(x)); Exp/Ln emissions are batched across
        # branches (with mm_ps slot-freeing order) to minimize table loads.
        dt_map = {}
        u_map = {}
        bcs_map = {}
        dbc_map = {}
        for br in range(3):
            xs = xs_map[br]
            dbc = mm_ps.tile([36, L2], f32, tag="mm")
            xw = c128b[:, br * 36:(br + 1) * 36]
            for c0 in range(0, L2, 512):
                nc.tensor.matmul(dbc[:, c0:c0 + 512], lhsT=xw,
                                 rhs=xs[:, c0:c0 + 512], start=True, stop=True)
            dbc36 = dbcp.tile([36, L2], bf16, tag="dbc36", name=f"dbc36_{br}")
            nc.scalar.copy(out=dbc36[:, :], in_=dbc[:, :])
            dbc_map[br] = dbc36
            # B/C rows to DRAM for partition-broadcast
            bcs = dramp.tile([32, L2], bf16, tag="bcs", name=f"bcs_{br}")
            nc.sync.dma_start(out=bcs[:, :], in_=dbc36[4:36, :])
            bcs_map[br] = bcs

        dtps_map = {}

        def mm_dt(br):
            dtps = mm_ps.tile([128, L2], f32, tag="mm", name=f"dtps_{br}")
            dw = c4[:, br * 128:(br + 1) * 128]
            for c0 in range(0, L2, 512):
                nc.tensor.matmul(dtps[:, c0:c0 + 512], lhsT=dw,
                                 rhs=dbc_map[br][0:4, c0:c0 + 512],
                                 start=True, stop=True)
            dtps_map[br] = dtps

        def dt_exp(br):
            nc.scalar.activation(dtps_map[br][:, :], dtps_map[br][:, :], AF.Exp,
                                 bias=c128f[:, PF_DTB + br:PF_DTB + br + 1])

        def dt_ln(br):
            dt = dtp.tile([128, L2], bf16, tag="dt", name=f"dt_{br}")
            nc.scalar.activation(dt[:, :], dtps_map[br][:, :], AF.Ln,
                                 bias=c128f[:, 75:76])
            dt_map[br] = dt

        # order dodges mm_ps (2 slots) deadlock with minimal Exp<->Ln switches
        mm_dt(0); mm_dt(1); dt_exp(0); dt_exp(1)
        dt_ln(0); mm_dt(2); dt_exp(2); dt_ln(1); dt_ln(2)

        for br in range(3):
            u = up.tile([128, L2], bf16, tag="u", name=f"u_{br}")
            nc.gpsimd.tensor_tensor(out=u[:, :], in0=dt_map[br][:, :],
                                    in1=xs_map[br][:, :], op=AL.mult)
            u_map[br] = u

        # ---- stage D2: powers, dBu, scan (custom), y ----
        y_sum = ysp.tile([128, L2], bf16)
        for br in range(3):
            img_i = 0 if br < 2 else 1
            xs = xs_map[br]
            dt = dt_map[br]
            u = u_map[br]
            bcs = bcs_map[br]

            dAq = [dAp.tile([128, NQ, TS2], bf16, tag="dA",
                            name=f"dA_{br}_{j}") for j in range(4)]
            # dA_s = exp(A_s * dt) directly on ACT (2x bf16); A_s packed cols
            for s in range(16):
                qi, si = s // NQ, s % NQ
                Ac = c128f[:, PF_A + br * 16 + s: PF_A + br * 16 + s + 1]
                nc.scalar.activation(dAq[qi][:, si, 0:L2], dt[:, :], AF.Exp,
                                     scale=Ac)
            for qi in range(4):
                nc.vector.memset(dAq[qi][:, :, L2:TS2], 0.0)

            uap = u[:, :]
            u_bc = bass.AP(tensor=uap.tensor, offset=uap.offset,
                           ap=[list(uap.ap[0]), [0, NQ], list(uap.ap[-1])])

            def bbc_dma(qi):
                t = bcp.tile([128, NQ, L2], bf16, tag="bc", name=f"bbc{br}_{qi}")
                srcf = bcs[qi * NQ:(qi + 1) * NQ, :].rearrange("a t -> (a t)")
                src_ap = bass.AP(tensor=srcf.tensor, offset=srcf.offset,
                                 ap=[[0, 128], [L2, NQ], [1, L2]])
                nc.sync.dma_start(out=t[:, :, :], in_=src_ap)
                return t

            # single dBu buffer (consumed serially on DVE); tails zeroed once
            dBu = dBup.tile([128, NQ, TS2], bf16, tag="dBu")
            nc.vector.memset(dBu[:, :, L2:TS2], 0.0)
            # B broadcasts prefetched two quarters ahead (each ~10us wall)
            bbcs = [bbc_dma(0), bbc_dma(1)]
            for qi in range(4):
                bbc = bbcs[qi]
                nc.vector.tensor_tensor(out=dBu[:, :, 0:L2], in0=u_bc,
                                        in1=bbc[:, :, :], op=AL.mult)
                if qi < 2:
                    bbcs.append(bbc_dma(qi + 2))
                # C broadcast prefetch (transfers while the scan runs)
                cbc = bcp.tile([128, NQ, L2], bf16, tag="bc")
                srcf2 = bcs[16 + qi * NQ:16 + (qi + 1) * NQ, :].rearrange(
                    "a t -> (a t)")
                src_ap2 = bass.AP(tensor=srcf2.tensor, offset=srcf2.offset,
                                  ap=[[0, 128], [L2, NQ], [1, L2]])
                nc.scalar.dma_start(out=cbc[:, :, :], in_=src_ap2)
                # scan in-place over dA quarter: h = dA*h_2back + dBu
                flat_a = dAq[qi][:, :, :].rearrange("p s t -> p (s t)")
                flat_b = dBu[:, :, :].rearrange("p s t -> p (s t)")
                nc.vector._custom_dve(SCAN_I2, out=flat_a, in0=flat_a,
                                      in1=flat_b)
                nc.vector.tensor_tensor(out=dAq[qi][:, :, 0:L2],
                                        in0=dAq[qi][:, :, 0:L2],
                                        in1=cbc[:, :, :], op=AL.mult)
                # fold this quarter straight into yb so its tile frees now
                # (keeps next unit's dA exps from stalling on dAp slots)
                nc.vector.tensor_tensor(out=dAq[qi][:, 0, 0:L2],
                                        in0=dAq[qi][:, 0, 0:L2],
                                        in1=dAq[qi][:, 2, 0:L2], op=AL.add)
                nc.vector.tensor_tensor(out=dAq[qi][:, 1, 0:L2],
                                        in0=dAq[qi][:, 1, 0:L2],
                                        in1=dAq[qi][:, 3, 0:L2], op=AL.add)
                if qi == 0:
                    yb = seqp.tile([128, L2], bf16, tag="yb")
                    nc.vector.tensor_tensor(out=yb[:, :],
                                            in0=dAq[0][:, 0, 0:L2],
                                            in1=dAq[0][:, 1, 0:L2], op=AL.add)
                else:
                    nc.vector.tensor_tensor(out=yb[:, :], in0=yb[:, :],
                                            in1=dAq[qi][:, 0, 0:L2], op=AL.add)
                    nc.vector.tensor_tensor(out=yb[:, :], in0=yb[:, :],
                                            in1=dAq[qi][:, 1, 0:L2], op=AL.add)
            # gate + accumulate (gpsimd)
            t1 = gatep.tile([128, L2], bf16, tag="t1")
            nc.vector.scalar_tensor_tensor(
                out=t1[:, :], in0=xs[:, :],
                scalar=c128f[:, PF_D + br:PF_D + br + 1],
                in1=yb[:, :], op0=AL.mult, op1=AL.add)
            zt = z[img_i]
            if br == 0:
                nc.gpsimd.tensor_tensor(out=y_sum[:, :], in0=t1[:, :],
                                        in1=zt[:, :], op=AL.mult)
            elif br == 1:
                t2 = gatep.tile([128, L2], bf16, tag="t2")
                nc.gpsimd.tensor_tensor(out=view3(t2[:, :]), in0=view3(t1[:, :]),
                                        in1=rev2(zt[:, :]), op=AL.mult)
                nc.gpsimd.tensor_tensor(out=view3(y_sum[:, :]),
                                        in0=view3(y_sum[:, :]),
                                        in1=rev2(t2[:, :]), op=AL.add)
            else:
                t2 = gatep.tile([128, L2], bf16, tag="t2")
                nc.vector.tensor_tensor(out=t2[:, :], in0=t1[:, :],
                                        in1=zt[:, :], op=AL.mult)
                nc.vector.tensor_tensor(out=y_sum[:, :], in0=y_sum[:, :],
                                        in1=t2[:, :], op=AL.add)

        # ---- final head ----
        att_sb = outp.tile([1, L2], bf16)
        lg = mm_ps.tile([1, L2], f32, tag="mm")
        for c0 in range(0, L2, 512):
            nc.tensor.matmul(lg[:, c0:c0 + 512], lhsT=vcolb[:, :],
                             rhs=y_sum[:, c0:c0 + 512], start=True, stop=True)
        nc.scalar.activation(att_sb[:, :], lg[:, :], AF.Sigmoid,
                             scale=0.5, bias=cmisc[0:1, 0:1])
        nc.sync.dma_start(out=att_out, in_=att_sb[:, :])

    nc.compile()
    return nc


def _pack_params(inputs):
    import ml_dtypes
    gi = lambda k: np.asarray(inputs[k], dtype=np.float32)

    p128f = np.zeros((128, PF_NCOL), np.float32)
    p128b = np.zeros((128, 108), np.float32)
    tags = ("f", "b", "s")
    for t, tag in enumerate(tags):
        p128f[:, PF_A + t * 16: PF_A + 16 + t * 16] = -np.exp(gi("A_log_" + tag))
        p128f[:, PF_CW + t * 4: PF_CW + 4 + t * 4] = gi("conv_w_" + tag)
        p128f[:, PF_CB + t] = gi("conv_b_" + tag)
        p128f[:, PF_DTB + t] = gi("dtproj_b_" + tag)
        p128f[:, PF_D + t] = gi("D_" + tag)
        p128b[:, t * 36:(t + 1) * 36] = gi("xproj_w_" + tag).T
    p128f[:, PF_V] = gi("out_proj_w").T @ gi("post_w")[0]
    p128f[:, 74] = LN_EPS
    p128f[:, 75] = 1.0
    ln_g, ln_b = gi("ln_g"), gi("ln_b")
    w1t = gi("in_proj_w").T
    w2t = gi("in_proj_s_w").T
    b1 = ln_b @ w1t
    b2 = ln_b @ w2t
    p128f[:, PF_BE + 0] = b1[0:128]
    p128f[:, PF_BE + 1] = b1[128:256]
    p128f[:, PF_BE + 2] = b2[0:128]
    p128f[:, PF_BE + 3] = b2[128:256]

    p17f = np.zeros((17, 64), np.float32)
    p17f[0:16] = gi("pre_w").T / 64.0
    p17f[16] = gi("pre_b")

    p4b = np.zeros((4, 384), np.float32)
    for t, tag in enumerate(tags):
        p4b[:, t * 128:(t + 1) * 128] = gi("dtproj_w_" + tag).T

    p64b = np.zeros((64, 512), np.float32)
    p64b[:, 0:256] = w1t * ln_g[:, None]
    p64b[:, 256:512] = w2t * ln_g[:, None]

    miscf = np.zeros((1, 2), np.float32)
    miscf[0, 0] = 0.5 * float(gi("post_b").reshape(-1)[0])

    bf = ml_dtypes.bfloat16
    return {
        "p128f": p128f,
        "p128b": p128b.astype(bf),
        "p17f": p17f.astype(bf),
        "p4b": p4b.astype(bf),
        "p64b": p64b.astype(bf),
        "miscf": miscf,
    }


def get_nc():
    if "nc" not in _cached:
        _cached["nc"] = _build_nc()
    return _cached["nc"]


def make_in_maps(inputs):
    params = _pack_params(inputs)
    img1 = np.ascontiguousarray(np.asarray(inputs["img1_features"], np.float32))
    img2 = np.ascontiguousarray(np.asarray(inputs["img2_features"], np.float32))
    in_maps = []
    for c in range(NCORES):
        m = dict(params)
        m["img1"] = np.ascontiguousarray(img1[c * BL:(c + 1) * BL])
        m["img2"] = np.ascontiguousarray(img2[c * BL:(c + 1) * BL])
        in_maps.append(m)
    return in_maps


def kernel(**inputs):
    from concourse.bass_utils import run_bass_kernel_spmd

    nc = get_nc()
    in_maps = make_in_maps(inputs)
    res = run_bass_kernel_spmd(nc, in_maps, core_ids=list(range(NCORES)))
    outs = []
    for r in res.results:
        att_i = r["att"].astype(np.float32).reshape(L, BL)  # col = 2t+b
        outs.append(att_i.T)                # (BL, L)
    att = np.concatenate(outs, axis=0) + 1e-6
    return att.reshape(B, C, 1, 1).astype(np.float32)
